# revision 1
# baseline (speedup 1.0000x reference)
"""Trainium2 Bass kernel for nn_MultiHeadAttention (B=4, T=2048, D=1024, H=16).

Sharding: tensor-parallel over heads - 2 heads per core on 8 cores.
Per core: QKV projections for its 2 heads, causal flash-style attention with
scores kept transposed (s^T[t_k, t_q]); softmax denominator from a ones-column
appended to V; causal mask applied as a 0/1 multiply on the exp output (DVE).

Emission-order software pipelining (engine queues are strict FIFO):
 - scores for group g+1 are emitted before att-accumulate of group g, so the
   PE has work while the Exp activations (the ACT engine is the P2
   bottleneck) of group g run;
 - one QKV-projection block of batch b+1 is emitted into each q-block of
   batch b's attention, filling PE during ACT-bound stretches;
 - a per-batch AllToAll (bf16, [8,128,256] per core) reshards head-split ->
   token-split while later batches compute; the output projection of batch b
   is emitted inside batch b+1's attention so the PE never stalls on the
   collective. Core c ends up owning tokens [256c, 256c+256) of every batch.

kernel(**inputs) takes the full unsharded inputs and returns the full output.
"""
import os
import sys
sys.path.insert(0, "/opt/trn_rl_repo")
os.environ.setdefault("JAX_PLATFORMS", "axon,cpu")
import numpy as np

B, T, D, H = 4, 2048, 1024, 16
DK = D // H           # 64
NCORES = 8
HPC = H // NCORES     # 2 heads per core
TB = B * T            # 8192 tokens
TS = TB // NCORES     # 1024 tokens per core in output projection
NKT = D // 128        # 8 contraction k-tiles


def build_nc(repeat=1, with_collective=True, parts="all", mdt="bf16", a2ag=1,
             xfac=None, bcast=False):
    import concourse.bacc as bacc
    import concourse.tile as tile
    import concourse.mybir as mybir

    f32 = mybir.dt.float32
    f32r = mybir.dt.bfloat16 if mdt == "bf16" else mybir.dt.float32r
    bf16 = mybir.dt.bfloat16
    AF = mybir.ActivationFunctionType

    nc = bacc.Bacc("TRN2", target_bir_lowering=False, debug=False,
                   num_devices=NCORES)
    xfac = {**{"exp": 1, "score": 1, "proj": 1, "att": 1,
               "mask": 1, "norm": 1, "xdma": 1}, **(xfac or {})}

    # ---- I/O ----
    xt = nc.dram_tensor("xt", [D, TB], f32r, kind="ExternalInput")
    wqkv = nc.dram_tensor("wqkv", [128, NKT, 3 * 128], f32r, kind="ExternalInput")
    bqkv = nc.dram_tensor("bqkv", [128, 3], f32, kind="ExternalInput")
    trif = nc.dram_tensor("trif", [128, 2, 512], bf16, kind="ExternalInput")
    idr = nc.dram_tensor("idr", [128, 128], f32r, kind="ExternalInput")
    wot = nc.dram_tensor("wot", [128, NKT, D], bf16, kind="ExternalInput")
    bov = nc.dram_tensor("bov", [1, D], bf16, kind="ExternalInput")
    onesd = nc.dram_tensor("onesd", [128, 16], f32r, kind="ExternalInput")
    onesbf = nc.dram_tensor("onesbf", [1, 128], bf16, kind="ExternalInput")
    y = nc.dram_tensor("y", [TS, D], f32, kind="ExternalOutput")

    # per-group collective buffers (plain DRAM tensors; pool tiles crash A2A);
    # a2ag = batches per AllToAll (1, 2, or 4)
    ngrp = B // a2ag
    agin = [nc.dram_tensor(f"agin{g}", [NCORES, 128, a2ag, 256], bf16)
            for g in range(ngrp)]
    agout = [nc.dram_tensor(f"agout{g}", [NCORES, 128, a2ag, 256], bf16)
             for g in range(ngrp)]
    split3 = (a2ag == 1 and parts == "all")
    if split3:
        agin3 = [nc.dram_tensor(f"agin3h{i}", [NCORES, 128, 128], bf16)
                 for i in range(2)]
        agout3 = [nc.dram_tensor(f"agout3h{i}", [NCORES, 128, 128], bf16)
                  for i in range(2)]

    xt_r = xt.ap().rearrange("(k p) t -> p k t", p=128)

    with tile.TileContext(nc) as tc:
        with tc.tile_pool(name="const", bufs=1) as cpool, \
             tc.tile_pool(name="xin", bufs=3) as xpool, \
             tc.tile_pool(name="qkv", bufs=2) as qkvpool, \
             tc.tile_pool(name="vt", bufs=2) as vtpool, \
             tc.tile_pool(name="st", bufs=6) as stpool, \
             tc.tile_pool(name="att", bufs=2) as attpool, \
             tc.tile_pool(name="yh", bufs=2) as yhpool, \
             tc.tile_pool(name="ysb", bufs=2) as ypool, \
             tc.tile_pool(name="small", bufs=4) as smpool, \
             tc.tile_pool(name="ps", bufs=2, space="PSUM") as ps, \
             tc.tile_pool(name="ps1", bufs=2, space="PSUM") as ps1, \
             tc.tile_pool(name="psat", bufs=2, space="PSUM") as psat:

            # ---- constants needed immediately ----
            wqkv_sb = cpool.tile([128, NKT, 3 * 128], f32r, tag="wqkv")
            nc.sync.dma_start(wqkv_sb[:, :, 0:128], wqkv.ap()[:, :, 0:128])
            bqkv_sb = cpool.tile([128, 3], f32, tag="bqkv")
            nc.sync.dma_start(bqkv_sb[:], bqkv.ap())
            nc.sync.dma_start(wqkv_sb[:, :, 128:384], wqkv.ap()[:, :, 128:384])
            # tiles for deferred constants (DMAs issued after the first xin
            # load so it isn't stuck behind them on the SP queue)
            tri_sb = cpool.tile([128, 2, 512], bf16, tag="tri")
            idn = cpool.tile([128, 128], f32 if mdt == "f32r" else f32r, tag="idn")
            wot_sb = cpool.tile([128, NKT, D], bf16, tag="wot")
            bov_sb = cpool.tile([1, D], bf16, tag="bov")
            ones16 = cpool.tile([128, 16], f32r, tag="ones16")
            ones_sb = cpool.tile([1, 128], f32r, tag="ones")
            onesb16 = cpool.tile([1, 128], bf16, tag="onesb")

            def deferred_consts_p2():
                nc.sync.dma_start(tri_sb[:], trif.ap())
                nc.sync.dma_start(
                    idn[:], idr.ap() if mdt == "bf16" else idr.ap().bitcast(f32))
                nc.sync.dma_start(ones16[:], onesd.ap())
                one_row = onesd.ap()[0:8, :].rearrange("p a -> (p a)").rearrange(
                    "(a b) -> a b", a=1)
                nc.sync.dma_start(ones_sb[:], one_row)
                nc.sync.dma_start(onesb16[:], onesbf.ap())

            def deferred_consts_p3():
                nc.sync.dma_start(wot_sb[:], wot.ap())
                nc.sync.dma_start(bov_sb[:], bov.ap())

            qkv = {}     # batch -> (qT, kT, v_b)
            attTs = {}   # batch -> attT tile

            def p1_stage(b, first=False):
                """Generator: one QKV-projection block per resume (4 total);
                the ones-columns of v are written before the last yield."""
                t0b = b * T
                qT = qkvpool.tile([128, T], f32r, tag="qT", name=f"qT{b}")
                kT = qkvpool.tile([128, T], f32r, tag="kT", name=f"kT{b}")
                v_b = qkvpool.tile([128, 16, 130], f32r, tag="v", name=f"v{b}")
                qkv[b] = (qT, kT, v_b)
                for blk in range(4):
                    t0 = t0b + blk * 512
                    xin = xpool.tile([128, NKT, 512], f32r, tag="xin",
                                     name=f"xin{b}_{blk}")
                    for _ in range(xfac["xdma"]):
                        nc.sync.dma_start(xin[:], xt_r[:, :, t0:t0 + 512])
                    if first and blk == 0:
                        deferred_consts_p2()
                        deferred_consts_p3()
                    vT = vtpool.tile([128, 512],
                                     f32 if mdt == "f32r" else f32r,
                                     tag="vT", name=f"vT{b}_{blk}")
                    for pi, dest in ((0, qT), (1, kT), (2, vT)):
                        pt = ps1.tile([128, 512], f32, tag="p1",
                                      name=f"pj{b}_{blk}_{pi}")
                        for _ in range(xfac["proj"]):
                            for kk in range(NKT):
                                nc.tensor.matmul(
                                    pt[:],
                                    wqkv_sb[:, kk, pi * 128:pi * 128 + 128],
                                    xin[:, kk, :],
                                    start=(kk == 0), stop=(kk == NKT - 1))
                        nc.vector.tensor_scalar_add(
                            dest[:, blk * 512:blk * 512 + 512] if pi < 2
                            else dest[:, 0:512],
                            pt[:], bqkv_sb[:, pi:pi + 1])
                    # transpose v^T[128, 512] -> v tiles [t, 130]
                    pt = ps1.tile([128, 512], f32 if mdt == "f32r" else f32r,
                                  tag="p1", name=f"ptr{b}_{blk}")
                    for tt in range(4):
                        nc.tensor.transpose(pt[:, tt * 128:tt * 128 + 128],
                                            vT[:, tt * 128:tt * 128 + 128],
                                            idn[:])
                    nc.vector.tensor_copy(
                        v_b[:, blk * 4:blk * 4 + 4, :].rearrange(
                            "p t (a c) -> p t a c", a=2)[:, :, :, 0:64],
                        pt[:, 0:512].rearrange("p (t a c) -> p t a c",
                                               t=4, a=2))
                    if blk == 3:
                        nc.vector.tensor_copy(
                            v_b[:, :, 64:65],
                            ones16[:].rearrange("p (a c) -> p a c", c=1))
                        nc.vector.tensor_copy(
                            v_b[:, :, 129:130],
                            ones16[:].rearrange("p (a c) -> p a c", c=1))
                    yield

            def scores_group(b, qblk, g, c0s, qT, kT):
                """Emit score matmuls + exp + causal mask for one group;
                returns stg tiles."""
                q0 = qblk * 512
                pt = []
                stg = stpool.tile([128, HPC, 2, 512], f32r, tag="st",
                                  name=f"st{b}_{qblk}_{g[0]}")
                for h in range(HPC):
                    qh = qT[64 * h:64 * h + 64, :]
                    kh = kT[64 * h:64 * h + 64, :]
                    pth = ps.tile([128, 1024], f32, tag="sc",
                                  name=f"sc{b}_{h}_{qblk}_{g[0]}")
                    pt.append(pth)
                    for jj, j in enumerate(g):
                        c0, c0a = c0s[j]
                        for _ in range(xfac["score"]):
                            nc.tensor.matmul(
                                pth[:, jj * 512 + c0a: jj * 512 + 512],
                                kh[:, 128 * j:128 * j + 128],
                                qh[:, q0 + c0a: q0 + 512],
                                start=True, stop=True)
                spans = []
                for jj, j in enumerate(g):
                    s, e = jj * 512 + c0s[j][1], (jj + 1) * 512
                    if spans and spans[-1][1] == s:
                        spans[-1][1] = e
                    else:
                        spans.append([s, e])
                for h in range(HPC):
                    stf = stg[:, h, :, :].rearrange("p a b -> p (a b)")
                    for s, e in spans:
                        for _ in range(xfac["exp"]):
                            nc.scalar.activation(stf[:, s:e], pt[h][:, s:e],
                                                 AF.Exp, scale=0.125)
                # causal mask: zero exp where key > query; one DVE 0/1
                # multiply covers both heads (h-strided AP)
                for jj, j in enumerate(g):
                    c0, c0a = c0s[j]
                    if 128 * j >= q0:
                        sl = stg[:, :, jj, c0a:512]
                        for _ in range(xfac["mask"]):
                            nc.vector.tensor_mul(
                                sl, sl, tri_sb[:, :, 0:512 - c0a])
                return stg

            def att_group(g, c0s, at, stg, njt, v_b):
                for h in range(HPC):
                    for xf in range(xfac["att"]):
                        for jj, j in enumerate(g):
                            c0, c0a = c0s[j]
                            nc.tensor.matmul(at[h][:, c0a:512],
                                             v_b[:, j, 65 * h:65 * h + 65],
                                             stg[:, h, jj, c0a:512],
                                             start=(j == 0 and
                                                    xf == xfac["att"] - 1),
                                             stop=(j == njt - 1 and
                                                   xf == xfac["att"] - 1))

            def p2_stage(b):
                """Generator: one q-block per resume; yields right before the
                normalize so the caller can wedge PE filler work in."""
                qT, kT, v_b = qkv[b]
                attT = attpool.tile([128, T], bf16, tag="attT", name=f"attT{b}")
                attTs[b] = attT
                for qblk in range(4):
                    njt = 4 * qblk + 4
                    q0 = qblk * 512
                    groups = [[i, i + 1] for i in range(0, njt, 2)]
                    c0s = {j: (max(0, 128 * j - q0),
                               max(0, 128 * j - q0))
                           for j in range(njt)}
                    at = [psat.tile([65, 512], f32, tag="at",
                                    name=f"at{b}_{qblk}_{hh}")
                          for hh in range(HPC)]
                    prev = None
                    for g in groups:
                        stg = scores_group(b, qblk, g, c0s, qT, kT)
                        if prev is not None:
                            att_group(prev[0], c0s, at, prev[1], njt, v_b)
                        prev = (g, stg)
                        if parts == "p12se":
                            nc.vector.tensor_copy(attT[:, q0:q0 + 512],
                                                  stg[:, 0, 0, :])
                            prev = None
                    if prev is not None:
                        att_group(prev[0], c0s, at, prev[1], njt, v_b)
                    yield
                    if parts == "p12se":
                        continue
                    # normalize: reciprocal of denom row, then broadcast
                    # (gpsimd partition_broadcast or PE K=1 ones-matmul)
                    for h in range(HPC * xfac["norm"]):
                        h = h % HPC
                        bc_sb = smpool.tile([64, 512], f32, tag="bcsb")
                        if bcast:
                            recip = smpool.tile([1, 512], f32, tag="recip")
                            nc.vector.reciprocal(recip[:], at[h][64:65, :])
                            nc.gpsimd.partition_broadcast(bc_sb[:], recip[:],
                                                          channels=64)
                        else:
                            recip = smpool.tile([1, 512], f32r, tag="recip")
                            with nc.allow_low_precision(
                                    reason="low-prec recip feeds matmul"):
                                nc.vector.reciprocal(recip[:], at[h][64:65, :])
                            bc = ps1.tile([128, 512], f32, tag="p1",
                                          name=f"bc{b}_{qblk}_{h}")
                            nc.tensor.matmul(bc[0:64, :], ones_sb[0:1, 0:64],
                                             recip[:], start=True, stop=True)
                            nc.vector.tensor_copy(bc_sb[:], bc[0:64, :])
                        nc.vector.tensor_mul(
                            attT[64 * h:64 * h + 64, q0:q0 + 512],
                            at[h][0:64, :], bc_sb[:])

            def p3_batch(pb, src):
                yh = yhpool.tile([128, NKT, 256], bf16, tag="yh",
                                 name=f"yh{pb}")
                nc.sync.dma_start(
                    yh[:], src[:, :, pb % a2ag, :].rearrange("j p t -> p j t"))
                for tt in range(2):
                    y_sb = ypool.tile([128, D], f32, tag="y",
                                      name=f"ysb{pb}_{tt}")
                    for eb in range(2):
                        pt = ps1.tile([128, 512], f32, tag="p1",
                                      name=f"p3_{pb}_{tt}_{eb}")
                        for kk in range(NKT):
                            nc.tensor.matmul(
                                pt[:],
                                yh[:, kk, tt * 128:tt * 128 + 128],
                                wot_sb[:, kk, eb * 512:eb * 512 + 512],
                                start=(kk == 0), stop=False)
                        nc.tensor.matmul(pt[:], onesb16[0:1, 0:128],
                                         bov_sb[0:1, eb * 512:eb * 512 + 512],
                                         start=False, stop=True)
                        nc.vector.tensor_copy(y_sb[:, eb * 512:eb * 512 + 512],
                                              pt[:])
                    nc.sync.dma_start(
                        y.ap()[256 * pb + tt * 128: 256 * pb + tt * 128 + 128, :],
                        y_sb[:])

            def ship_half3(hb):
                """Ship half hb (tokens 1024*hb..+1024) of batch 3's attT;
                core c ends up owning tokens [128c, 128c+128) of the half."""
                attT = attTs[3]
                for j in range(NCORES):
                    nc.sync.dma_start(
                        agin3[hb].ap()[j],
                        attT[:, 1024 * hb + 128 * j: 1024 * hb + 128 * j + 128])
                if with_collective:
                    nc.gpsimd.collective_compute(
                        "AllToAll", mybir.AluOpType.bypass,
                        replica_groups=[list(range(NCORES))],
                        ins=[agin3[hb].ap().opt()],
                        outs=[agout3[hb].ap().opt()])
                    return agout3[hb].ap()
                return agin3[hb].ap()

            def p3_half3(hb, src):
                yh = yhpool.tile([128, NKT, 128], bf16, tag="yh",
                                 name=f"yh3h{hb}")
                nc.sync.dma_start(yh[:], src.rearrange("j p t -> p j t"))
                y_sb = ypool.tile([128, D], f32, tag="y", name=f"ysb3h{hb}")
                for eb in range(2):
                    pt = ps1.tile([128, 512], f32, tag="p1",
                                  name=f"p33_{hb}_{eb}")
                    for kk in range(NKT):
                        nc.tensor.matmul(
                            pt[:], yh[:, kk, :],
                            wot_sb[:, kk, eb * 512:eb * 512 + 512],
                            start=(kk == 0), stop=False)
                    nc.tensor.matmul(pt[:], onesb16[0:1, 0:128],
                                     bov_sb[0:1, eb * 512:eb * 512 + 512],
                                     start=False, stop=True)
                    nc.vector.tensor_copy(y_sb[:, eb * 512:eb * 512 + 512],
                                          pt[:])
                nc.sync.dma_start(
                    y.ap()[768 + 128 * hb: 768 + 128 * hb + 128, :], y_sb[:])

            def ship_attT(b):
                """attT -> agin slices; fire the group's A2A when complete."""
                attT = attTs[b]
                g = b // a2ag
                for j in range(NCORES):
                    nc.sync.dma_start(agin[g].ap()[j][:, b % a2ag, :],
                                      attT[:, 256 * j: 256 * j + 256])
                if parts != "all":
                    return None
                if b % a2ag != a2ag - 1:
                    return None
                if with_collective:
                    nc.gpsimd.collective_compute(
                        "AllToAll", mybir.AluOpType.bypass,
                        replica_groups=[list(range(NCORES))],
                        ins=[agin[g].ap().opt()], outs=[agout[g].ap().opt()])
                    return agout[g].ap()
                return agin[g].ap()  # timing variant: wrong data

            pending_p3 = []
            for rep in range(repeat):
                if rep == 0:
                    for _ in p1_stage(0, first=True):
                        pass
                for b in range(B):
                    if parts == "p1":
                        # timing variant: anchor P1 outputs, skip attention
                        qT, kT, v_b = qkv[b]
                        attT = attpool.tile([128, T], bf16, tag="attT",
                                            name=f"attT{b}")
                        attTs[b] = attT
                        nc.vector.tensor_copy(attT[:, 0:130], v_b[:, 0, :])
                        nc.vector.tensor_copy(attT[:, 256:768], qT[:, 0:512])
                        nc.vector.tensor_copy(attT[:, 1024:1536], kT[:, 0:512])
                        ship_attT(b)
                        if b + 1 < B:
                            for _ in p1_stage(b + 1):
                                pass
                        continue
                    p2 = p2_stage(b)
                    if b + 1 < B:
                        p1n = p1_stage(b + 1)
                    elif rep + 1 < repeat:
                        p1n = p1_stage(0)  # next rep's P1 as tail filler
                    else:
                        p1n = iter(())
                    half_src = None
                    # next batch's P1 filler weighted toward the exp-heavy
                    # late q-blocks (ACT load grows ~linearly with qblk)
                    for qblk, nfill in zip(range(4), (0, 1, 1, 2)):
                        next(p2, None)
                        if qblk == 2 and pending_p3:
                            pending_p3.pop(0)()
                        for _ in range(nfill):
                            next(p1n, None)
                        if split3 and b == 3 and qblk == 2:
                            # qblk1's normalize was emitted at the top of this
                            # resume: tokens 0..1023 of batch 3 are final
                            half_src = ship_half3(0)
                    if split3 and b == 3 and half_src is not None:
                        if pending_p3:   # prior rep's deferred tail
                            pending_p3.pop(0)()
                        p3_half3(0, half_src)
                    for _ in p2:
                        pass
                    for _ in p1n:
                        pass
                    if split3 and b == 3:
                        hs1 = ship_half3(1)
                        pending_p3.append(
                            lambda hs1=hs1: p3_half3(1, hs1))
                    else:
                        src = ship_attT(b)
                        if src is not None:
                            pending_p3.extend(
                                (lambda bb=bb, src=src: p3_batch(bb, src))
                                for bb in range((b // a2ag) * a2ag, b + 1))
            for fn in pending_p3:
                fn()
    nc.compile()
    return nc


# ------------------------------------------------------------------
# Host-side wrapper
# ------------------------------------------------------------------
_CACHE = {}


def _prep_inputs(x, wq, bq, wk, bk, wv, bv, wo, bo, mdt="bf16"):
    import ml_dtypes
    if mdt == "bf16":
        cast = lambda a: np.asarray(a, ml_dtypes.bfloat16)
    else:
        cast = lambda a: np.asarray(a, np.float32)
    bcast = lambda a: np.asarray(a, ml_dtypes.bfloat16)
    xt = np.ascontiguousarray(x.reshape(TB, D).T)          # [D, TB]

    def lhsT_pack(W):   # W [128, D] -> [128p, NKT, 128m]
        return np.ascontiguousarray(W.T.reshape(NKT, 128, 128).transpose(1, 0, 2))

    # triangular 0/1 mask constant (0 = masked-out key>query, 1 = keep);
    # both rows hold the same pattern so one [128, 2, w] AP covers both heads
    trif = np.zeros((128, 2, 512), np.float32)
    cols = np.arange(512)
    for p in range(128):
        trif[p, 0, :] = np.where(cols < p, 0.0, 1.0)
        trif[p, 1, :] = trif[p, 0, :]

    wott = np.ascontiguousarray(wo.T.reshape(NKT, 128, D).transpose(1, 0, 2))
    bov = bo.reshape(1, D).astype(np.float32)
    idr = np.eye(128, dtype=np.float32)

    in_maps = []
    for c in range(NCORES):
        h0, h1 = HPC * c, HPC * c + 1
        Wq = np.concatenate([wq[h0], wq[h1]], axis=0)      # [128, D]
        Wk = np.concatenate([wk[h0], wk[h1]], axis=0)
        Wv = np.concatenate([wv[h0], wv[h1]], axis=0)
        wqkvp = np.concatenate([lhsT_pack(Wq), lhsT_pack(Wk), lhsT_pack(Wv)],
                               axis=2)                     # [128, NKT, 384]
        bqkvp = np.stack([np.concatenate([bq[h0], bq[h1]]),
                          np.concatenate([bk[h0], bk[h1]]),
                          np.concatenate([bv[h0], bv[h1]])], axis=1)  # [128,3]
        in_maps.append({
            "xt": cast(xt),
            "wqkv": cast(np.ascontiguousarray(wqkvp, np.float32)),
            "bqkv": np.ascontiguousarray(bqkvp, np.float32),
            "trif": bcast(trif),
            "idr": cast(idr),
            "wot": bcast(wott),
            "bov": bcast(bov),
            "onesd": cast(np.ones((128, 16), np.float32)),
            "onesbf": bcast(np.ones((1, 128), np.float32)),
        })
    return in_maps


MDT = "bf16"   # matmul dtype: "bf16" or "f32r"
A2AG = 1       # batches per AllToAll


def kernel(x, wq, bq, wk, bk, wv, bv, wo, bo):
    from concourse import bass_utils
    x, wq, bq, wk, bk, wv, bv, wo, bo = (
        np.asarray(a, np.float32) for a in (x, wq, bq, wk, bk, wv, bv, wo, bo))
    if "nc" not in _CACHE:
        _CACHE["nc"] = build_nc(mdt=MDT, a2ag=A2AG)
    nc = _CACHE["nc"]
    in_maps = _prep_inputs(x, wq, bq, wk, bk, wv, bv, wo, bo, mdt=MDT)
    res = bass_utils.run_bass_kernel_spmd(nc, in_maps, core_ids=list(range(NCORES)))
    ys = np.stack([res.results[c]["y"] for c in range(NCORES)])  # [8, 1024, D]
    return assemble(ys)


def assemble(ys):
    """Per-core y rows -> full [B, T, D] output.
    Batches 0-2: core c holds tokens [256c, 256c+256) at rows 256b+.
    Batch 3 (tail-split halves): rows 768:896 = tokens [128c, 128c+128),
    rows 896:1024 = tokens [1024+128c, 1024+128c+128)."""
    out = np.empty((B, T, D), np.float32)
    for c in range(NCORES):
        for b in range(3):
            out[b, 256 * c:256 * c + 256] = ys[c][256 * b:256 * b + 256]
        out[3, 128 * c:128 * c + 128] = ys[c][768:896]
        out[3, 1024 + 128 * c:1024 + 128 * c + 128] = ys[c][896:1024]
    return out

